# revision 34
# baseline (speedup 1.0000x reference)
"""AttentionBlock Trainium2 Bass kernel.

Full-input contract: kernel(**inputs) takes the complete tensors from
setup_inputs() and returns the full (4, 256, 64, 64) float32 output.

Sharding: 8 cores = 4 batches x 2 query-token halves. Each core:
  - group-norms its batch image (stats over all 4096 tokens),
  - computes k, v for all tokens, q for its 2048 local tokens,
  - attention (4 heads, exact softmax without max-subtraction: scores ~N(0,1)),
  - output projection + residual for its token half.
Host side only slices/concats (token order within a batch is rolled so the
local half is always first -> the same SPMD program runs on every core).

Schedule notes (from NTFF traces): the steady-state attention loop is
ACT-bound (exp of 33.5M scores/core at ~1 elem/cycle/lane). Everything else
is arranged to keep the exp stream start as early as possible and the
pre/post phases lean:
  - GN stats split across ACT (sumsq via Square+accum) and DVE (sum via
    tensor_reduce) so the two passes run concurrently.
  - K/Q projections (needed by the first scores) come before V.
  - V bias is folded into the host-precomputed bp' = bp + wp @ bv (softmax
    rows sum to 1), so V-proj is 2 matmuls/chunk.
  - Per-(qb,h) normalization happens inside the loop (the 1/Z broadcast
    matmul borrows the avp PSUM bank right after it is evacuated), so the
    tail is just the output projection.
"""

import sys

sys.path.insert(0, "/opt/trn_rl_repo")

import numpy as np

# hardcoded problem geometry
B, C, H, W = 4, 256, 64, 64
NTOK = H * W            # 4096 keys per image
NLOC = NTOK // 2        # 2048 queries per core
QB = 1024               # query block (scores psum tile free size)
HEADS, D = 4, 64
GROUPS, CPG = 8, 32     # 8 groups x 32 channels
EPS = 1e-5
NCH = 32                # key chunks of 128
VSEG = HEADS * (D + 1)  # 260: per-chunk stride in the VT buffer

_cached = {}


def _build_nc():
    import concourse.mybir as mybir
    import concourse.tile as tile
    from concourse import bacc
    from concourse.bass import ds, ts

    fp32 = mybir.dt.float32
    bf16 = mybir.dt.bfloat16
    AF = mybir.ActivationFunctionType
    OP = mybir.AluOpType
    AX = mybir.AxisListType

    nc = bacc.Bacc("TRN2", target_bir_lowering=False, debug=False, num_devices=8)

    xb = nc.dram_tensor("xb", [C, NTOK], fp32, kind="ExternalInput").ap()
    wqT = nc.dram_tensor("wqT", [C, C], fp32, kind="ExternalInput").ap()
    wkT = nc.dram_tensor("wkT", [C, C], fp32, kind="ExternalInput").ap()
    wvT = nc.dram_tensor("wvT", [C, C], fp32, kind="ExternalInput").ap()
    wpT = nc.dram_tensor("wpT", [C, C], fp32, kind="ExternalInput").ap()
    bqc = nc.dram_tensor("bqc", [C, 1], fp32, kind="ExternalInput").ap()
    bkc = nc.dram_tensor("bkc", [C, 1], fp32, kind="ExternalInput").ap()
    bpc2 = nc.dram_tensor("bpc2", [C, 1], fp32, kind="ExternalInput").ap()
    gnw = nc.dram_tensor("gnw", [C, 1], fp32, kind="ExternalInput").ap()
    gnb = nc.dram_tensor("gnb", [C, 1], fp32, kind="ExternalInput").ap()
    yo = nc.dram_tensor("y", [C, NLOC], fp32, kind="ExternalOutput").ap()

    from contextlib import ExitStack

    with tile.TileContext(nc) as tc, ExitStack() as ctx:
        pool = lambda name, bufs: ctx.enter_context(tc.tile_pool(name=name, bufs=bufs))
        # whole-kernel pools
        consts = pool("consts", 1)
        otp = pool("ot", 1)       # outT (2 x 4KB)
        xkp = pool("xk", 1)       # kept x tiles for the residual (2 x 8KB)
        xb2p = pool("xb2", 1)     # x + bp' residual base (2 x 8KB)

        # ---- x loads first (critical path), split for queue parallelism ----
        xh = {}
        for c2 in range(2):
            for hf in range(2):
                if hf == 0:
                    t = xkp.tile([128, NLOC], fp32, tag=f"xk{c2}",
                                 name=f"x{c2}h{hf}")
                else:
                    t = consts.tile([128, NLOC], fp32, tag=f"xt{c2}",
                                    name=f"x{c2}h{hf}")
                for q4 in range(4):
                    nc.sync.dma_start(
                        t[:, ds(q4 * 512, 512)],
                        xb[ts(c2, 128), ds(hf * NLOC + q4 * 512, 512)],
                    )
                xh[(c2, hf)] = t

        # ---- constants / weights ----
        ones_row = consts.tile([1, NLOC], bf16, tag="ones_row")
        nc.gpsimd.memset(ones_row[:], 1.0)
        ones_col = consts.tile([1, 128], bf16, tag="ones_col")
        nc.gpsimd.memset(ones_col[:], 1.0)
        eps4 = consts.tile([4, 1], fp32, tag="eps4")
        nc.gpsimd.memset(eps4[:], EPS)
        ones64f = consts.tile([1, D], fp32, tag="ones64f")
        nc.gpsimd.memset(ones64f[:], 1.0)
        # ones row AT partition 64: lhsT for the 1/Z broadcast matmul (the
        # Z row lives at partition D of oa/rz tiles; contraction partitions
        # of lhsT and rhs must match)
        onesP = consts.tile([D + 1, D], bf16, tag="onesP")
        nc.gpsimd.memset(onesP[:], 1.0)
        # mask4T[p, gl] = 1/(32*4096) if p//32 == gl: turns per-channel SUM
        # columns into per-group MEANs via one matmul
        mask4T = consts.tile([128, 4], fp32, tag="mask4T")
        nc.gpsimd.memset(mask4T[:], 0.0)
        for gl in range(4):
            nc.gpsimd.memset(
                mask4T[gl * CPG : (gl + 1) * CPG, gl : gl + 1],
                1.0 / (CPG * NTOK),
            )
        # mask4B[gl, p] = 1.0 if p//32 == gl  (group->channel broadcast);
        # row gl>0 starts at partition gl, which memset can't address -> DMA
        # a constant-1.0 fp32 row into place instead.
        mask4B = consts.tile([4, 128], fp32, tag="mask4B")
        nc.gpsimd.memset(mask4B[:], 0.0)
        for gl in range(4):
            nc.sync.dma_start(
                mask4B[gl : gl + 1, gl * CPG : (gl + 1) * CPG],
                ones64f[0:1, 0:CPG],
            )

        bcols = {}
        for nm, src in (("q", bqc), ("k", bkc), ("gw", gnw), ("gb", gnb),
                        ("p2", bpc2)):
            for k2 in range(2):
                t = consts.tile([128, 1], fp32, tag=f"b{nm}{k2}")
                nc.sync.dma_start(t[:], src[ts(k2, 128), :])
                bcols[(nm, k2)] = t

        # weights to bf16 (DVE idle at startup)
        wb = {}
        with tc.tile_pool(name="wload", bufs=4) as wldp:
            for nm, srcw in (("k", wkT), ("q", wqT), ("v", wvT), ("p", wpT)):
                for k2 in range(2):
                    t = wldp.tile([128, C], fp32, tag="wf",
                                  name=f"wf_{nm}{k2}")
                    nc.sync.dma_start(t[:], srcw[ts(k2, 128), :])
                    tb = consts.tile([128, C], bf16, tag=f"w{nm}b{k2}",
                                     name=f"w{nm}b{k2}")
                    nc.vector.tensor_copy(tb[:], t[:])
                    wb[(nm, k2)] = tb

        # VT: per key-chunk j, per head h: [vT(128,64) | ones] at col j*260+h*65
        VT = consts.tile([128, NCH * VSEG], bf16, tag="VT")
        vt_ones = VT[:].rearrange("p (j h x) -> p j h x", j=NCH, h=HEADS)[
            :, :, :, D : D + 1
        ]
        nc.gpsimd.memset(vt_ones, 1.0)

        with tc.tile_pool(name="kq", bufs=1) as kqpool:
            ksb = [kqpool.tile([128, NTOK], bf16, tag=f"ksb{m}", name=f"ksb{m}")
                   for m in range(2)]
            qsb = [kqpool.tile([128, NLOC], bf16, tag=f"qsb{m}", name=f"qsb{m}")
                   for m in range(2)]
            ksw = [kqpool.tile([128, NTOK], bf16, tag=f"ksw{m}", name=f"ksw{m}")
                   for m in range(2)]
            qsw = [kqpool.tile([128, NLOC], bf16, tag=f"qsw{m}", name=f"qsw{m}")
                   for m in range(2)]
            with tc.tile_pool(name="xn", bufs=1) as xnpool:
                xn = [xnpool.tile([128, NTOK], bf16, tag=f"xn{c2}", name=f"xn{c2}")
                      for c2 in range(2)]

                # ---- group-norm: sumsq on ACT (Square+accum), sum on DVE
                # (tensor_reduce) so both passes run concurrently.
                with tc.tile_pool(name="stat", bufs=2) as statp, \
                     tc.tile_pool(name="gnps", bufs=2, space="PSUM") as gnps:
                    for c2 in range(2):
                        sacc = statp.tile([128, 4], fp32, tag="sacc")
                        scr = statp.tile([128, NLOC], bf16, tag="scr", bufs=1)
                        for hf in range(2):
                            nc.scalar.activation(
                                scr[:], xh[(c2, hf)][:], AF.Square,
                                accum_out=sacc[:, 2 + hf : 3 + hf],
                            )
                            nc.vector.tensor_reduce(
                                sacc[:, hf : hf + 1], xh[(c2, hf)][:],
                                axis=AX.X, op=OP.add,
                            )
                        # me2: [sum_p, sumsq_p] (mask4T folds the 1/N)
                        me2 = statp.tile([128, 2], fp32, tag="me2")
                        nc.vector.tensor_add(
                            me2[:, 0:1], sacc[:, 0:1], sacc[:, 1:2]
                        )
                        nc.vector.tensor_add(
                            me2[:, 1:2], sacc[:, 2:3], sacc[:, 3:4]
                        )
                        # group [mean, E[x^2]] onto partitions 0-3 via mask MM
                        gmp = gnps.tile([4, 2], fp32, tag="gmp")
                        nc.tensor.matmul(gmp[:], mask4T[:], me2[:])
                        gmsb = statp.tile([4, 2], fp32, tag="gmsb")
                        nc.vector.tensor_copy(gmsb[:], gmp[:])
                        gvar = statp.tile([4, 1], fp32, tag="gvar")
                        nc.vector.tensor_tensor(
                            gvar[:], gmsb[:, 0:1], gmsb[:, 0:1], op=OP.mult
                        )
                        nc.vector.tensor_tensor(
                            gvar[:], gmsb[:, 1:2], gvar[:], op=OP.subtract
                        )
                        gstd = statp.tile([4, 1], fp32, tag="gstd")
                        nc.scalar.activation(gstd[:], gvar[:], AF.Sqrt,
                                             bias=eps4[:])
                        grstd = statp.tile([4, 1], fp32, tag="grstd")
                        nc.vector.reciprocal(grstd[:], gstd[:])
                        # broadcast group stats back to channel columns
                        rcolp = gnps.tile([128, 1], fp32, tag="rcolp")
                        nc.tensor.matmul(rcolp[:], mask4B[:], grstd[:])
                        mcolp = gnps.tile([128, 1], fp32, tag="mcolp")
                        nc.tensor.matmul(mcolp[:], mask4B[:], gmsb[:, 0:1])
                        acol = statp.tile([128, 1], fp32, tag="acol")
                        nc.vector.tensor_tensor(
                            acol[:], rcolp[:], bcols[("gw", c2)][:], op=OP.mult
                        )
                        bcol = statp.tile([128, 1], fp32, tag="bcol")
                        nc.vector.tensor_tensor(
                            bcol[:], mcolp[:], acol[:], op=OP.mult
                        )
                        nc.vector.tensor_tensor(
                            bcol[:], bcols[("gb", c2)][:], bcol[:], op=OP.subtract
                        )
                        for hf in range(2):
                            nc.vector.tensor_scalar(
                                xn[c2][:, ds(hf * NLOC, NLOC)], xh[(c2, hf)][:],
                                acol[:], bcol[:], op0=OP.mult, op1=OP.add,
                            )

                # ---- k, q projections first (they gate the first scores),
                # then the swapped copies, then v. The PSUM->SBUF move with
                # bias runs on ACT (Identity, per-partition bias) -- DVE is
                # the pre-phase bottleneck, ACT is idle here.
                with tc.tile_pool(name="qkps", bufs=2, space="PSUM") as qkps:
                    for m in range(2):
                        for t in range(4):
                            pk = qkps.tile([128, 1024], fp32, tag="pk")
                            for half in range(2):
                                for k2 in range(2):
                                    nc.tensor.matmul(
                                        pk[:, ds(half * 512, 512)],
                                        wb[("k", k2)][:, ts(m, 128)],
                                        xn[k2][:, ds(t * 1024 + half * 512, 512)],
                                        start=(k2 == 0), stop=(k2 == 1),
                                    )
                            nc.scalar.activation(
                                ksb[m][:, ds(t * 1024, 1024)], pk[:],
                                AF.Identity, bias=bcols[("k", m)][:],
                            )
                        for t in range(2):
                            pq = qkps.tile([128, 1024], fp32, tag="pk")
                            for half in range(2):
                                for k2 in range(2):
                                    nc.tensor.matmul(
                                        pq[:, ds(half * 512, 512)],
                                        wb[("q", k2)][:, ts(m, 128)],
                                        xn[k2][:, ds(t * 1024 + half * 512, 512)],
                                        start=(k2 == 0), stop=(k2 == 1),
                                    )
                            nc.scalar.activation(
                                qsb[m][:, ds(t * 1024, 1024)], pq[:],
                                AF.Identity, bias=bcols[("q", m)][:],
                            )
                        nc.sync.dma_start(ksw[m][0:64, :], ksb[m][64:128, :])
                        nc.sync.dma_start(ksw[m][64:128, :], ksb[m][0:64, :])
                        nc.sync.dma_start(qsw[m][0:64, :], qsb[m][64:128, :])
                        nc.sync.dma_start(qsw[m][64:128, :], qsb[m][0:64, :])
                    # v projection (bias folded into bp' on the host)
                    for j in range(NCH):
                        pv = qkps.tile([128, C], fp32, tag="pv", bufs=2)
                        nc.tensor.matmul(
                            pv[:], xn[0][:, ts(j, 128)], wb[("v", 0)][:],
                            start=True, stop=False,
                        )
                        nc.tensor.matmul(
                            pv[:], xn[1][:, ts(j, 128)], wb[("v", 1)][:],
                            start=False, stop=True,
                        )
                        dst = VT[:, ds(j * VSEG, VSEG)].rearrange(
                            "p (h x) -> p h x", h=HEADS
                        )[:, :, 0:D]
                        nc.vector.tensor_copy(
                            dst, pv[:].rearrange("p (h x) -> p h x", h=HEADS)
                        )

            # ---- attention (ACT-bound steady state) ----
            outT = [otp.tile([128, NLOC], bf16, tag=f"outT{m}", name=f"outT{m}")
                    for m in range(2)]
            with tc.tile_pool(name="esc", bufs=6) as escp, \
                 tc.tile_pool(name="oa", bufs=2) as oap, \
                 tc.tile_pool(name="rzp", bufs=2) as rzp, \
                 tc.tile_pool(name="tmpn", bufs=2) as tmpp, \
                 tc.tile_pool(name="ys", bufs=2) as ysp, \
                 tc.tile_pool(name="scps", bufs=3, space="PSUM") as scps, \
                 tc.tile_pool(name="avps", bufs=1, space="PSUM") as avps:
                def emit_av(avp, h, j, esc):
                    for t in range(2):
                        nc.tensor.matmul(
                            avp[:, ts(t, 512)],
                            VT[:, ds(j * VSEG + h * (D + 1), D + 1)],
                            esc[:, ts(t, 512)],
                            start=(j == 0), stop=(j == NCH - 1),
                        )

                def emit_norm(dn):
                    # 1/Z broadcast + scale for a finished block; emitted a
                    # few chunk-pairs into the NEXT block so the PE / ACT
                    # streams never stall at the block boundary. The zbc
                    # broadcast tile borrows a scps slot (its previous S
                    # tile's exp is long done by now).
                    dth, dhp, dqb, doa, drzc = dn
                    dzbc = scps.tile([D, QB], fp32, tag="sc", name="zbc")
                    for t in range(2):
                        nc.tensor.matmul(
                            dzbc[:, ts(t, 512)], ones_col[0:1, 0:D],
                            drzc[0:1, ds(t * 512, 512)],
                        )
                    if dhp == 0:
                        nc.vector.tensor_tensor(
                            outT[dth][0:D, ds(dqb * QB, QB)], doa[0:D, :],
                            dzbc[:], op=OP.mult,
                        )
                    else:
                        tm = tmpp.tile([D, QB], bf16, tag="tm")
                        nc.vector.tensor_tensor(tm[:], doa[0:D, :], dzbc[:],
                                                op=OP.mult)
                        nc.sync.dma_start(
                            outT[dth][64:128, ds(dqb * QB, QB)], tm[:]
                        )

                deferred = None
                carry = None
                for qb in range(2):
                    for h in range(HEADS):
                        th, hp = h // 2, h % 2
                        # block 0 only: filler tiles allocated BEFORE avp so
                        # the bufs=1 arena rotation stays in usage order;
                        # their matmuls are emitted into the first-iteration
                        # bubbles (PE waiting on the very first exps), which
                        # otherwise trip the HAM clock gate into half-rate
                        # for the next ~60us.
                        fillers = []
                        if qb == 0 and h == 0:
                            fillers = [
                                avps.tile([D + 1, QB], fp32, tag="av",
                                          name=f"fill{f}")
                                for f in range(4)
                            ]
                        avp = avps.tile([D + 1, QB], fp32, tag="av",
                                        name=f"avp{qb}{h}")
                        pending = []  # (j, esc) awaiting A@V matmuls
                        for jj in range(0, NCH, 2):
                            # chunk pair: even chunk from ksb/qsb at rows
                            # hp*64, odd chunk from the swapped copies at the
                            # OTHER row group -> the four score matmuls run
                            # concurrently in disjoint 64-row halves.
                            b0 = hp * 64
                            b1 = 64 - b0
                            S0 = scps.tile([128, QB], fp32, tag="sc",
                                           name="S0")
                            S1 = scps.tile([128, QB], fp32, tag="sc",
                                           name="S1")
                            # alternate the two 64-row groups every matmul so
                            # consecutive MMs touch disjoint array rows (row
                            # tiling can overlap their streams)
                            for t in range(2):
                                nc.tensor.matmul(
                                    S0[:, ts(t, 512)],
                                    ksb[th][b0 : b0 + 64, ts(jj, 128)],
                                    qsb[th][b0 : b0 + 64,
                                            ds(qb * QB + t * 512, 512)],
                                )
                                nc.tensor.matmul(
                                    S1[:, ts(t, 512)],
                                    ksw[th][b1 : b1 + 64, ts(jj + 1, 128)],
                                    qsw[th][b1 : b1 + 64,
                                            ds(qb * QB + t * 512, 512)],
                                )
                            # (t loop already alternates S0/S1 row groups)
                            esc0 = escp.tile([128, QB], bf16, tag="esc",
                                             name="esc0")
                            nc.scalar.activation(esc0[:], S0[:], AF.Exp,
                                                 scale=0.125)
                            esc1 = escp.tile([128, QB], bf16, tag="esc",
                                             name="esc1")
                            nc.scalar.activation(esc1[:], S1[:], AF.Exp,
                                                 scale=0.125)
                            # A@V lags two pairs behind: by the time the PE
                            # reaches these, their exps finished long ago, so
                            # the in-order PE queue never blocks on ACT.
                            if fillers and jj == 0:
                                # one dense ~8-MM burst right at attention
                                # entry: mirrors the block-boundary burst
                                # that reliably flips the HAM clock gate to
                                # full rate (small 2-MM fillers did not)
                                for ft in fillers:
                                    for t in range(2):
                                        nc.tensor.matmul(
                                            ft[:, ts(t, 512)],
                                            VT[:, 0 : D + 1],
                                            xn[0][:, ts(t, 512)],
                                        )
                            if jj == 0 and carry is not None:
                                # previous block's last two AV pairs: their
                                # exps completed while this block's first
                                # scores ran, so the PE never idles on them.
                                cavp, ch, cpend = carry
                                for pj, pesc in cpend:
                                    emit_av(cavp, ch, pj, pesc)
                                oa = oap.tile([D + 1, QB], fp32, tag="oa")
                                nc.vector.tensor_copy(oa[:], cavp[:])
                                zrow = rzp.tile([1, QB], fp32, tag="zrow",
                                                bufs=2)
                                nc.sync.dma_start(zrow[:], oa[D : D + 1, :])
                                rzf = rzp.tile([1, QB], fp32, tag="rzf",
                                               bufs=2)
                                nc.vector.reciprocal_approx_fast(
                                    rzf[:], zrow[:]
                                )
                                rzc = rzp.tile([1, QB], bf16, tag="rzc",
                                               bufs=2)
                                nc.vector.tensor_copy(rzc[:], rzf[:])
                                deferred = carry_meta + (oa, rzc)
                                carry = None
                            if jj == 8 and deferred is not None:
                                emit_norm(deferred)
                                deferred = None
                            if len(pending) >= 4:
                                for pj, pesc in pending[:2]:
                                    emit_av(avp, h, pj, pesc)
                                pending = pending[2:]
                            pending += [(jj, esc0), (jj + 1, esc1)]
                        # keep the last two pairs for the next block
                        carry = (avp, h, pending)
                        carry_meta = (th, hp, qb)
                    if qb == 0:
                        # residual base x + bp' computed in DVE slack during
                        # the qb=1 attention block
                        xb2 = []
                        for m in range(2):
                            x2 = xb2p.tile([128, NLOC], fp32, tag=f"xb2{m}",
                                           name=f"xb2{m}")
                            nc.vector.tensor_scalar_add(
                                x2[:], xh[(m, 0)][:], bcols[("p2", m)][:]
                            )
                            xb2.append(x2)
                # drain the last block: carried AV pairs, 1/Z, normalize
                cavp, ch, cpend = carry
                for pj, pesc in cpend:
                    emit_av(cavp, ch, pj, pesc)
                oa = oap.tile([D + 1, QB], fp32, tag="oa")
                nc.vector.tensor_copy(oa[:], cavp[:])
                zrow = rzp.tile([1, QB], fp32, tag="zrow", bufs=2)
                nc.sync.dma_start(zrow[:], oa[D : D + 1, :])
                rzf = rzp.tile([1, QB], fp32, tag="rzf", bufs=2)
                nc.vector.reciprocal_approx_fast(rzf[:], zrow[:])
                rzc = rzp.tile([1, QB], bf16, tag="rzc", bufs=2)
                nc.vector.tensor_copy(rzc[:], rzf[:])
                # qb0-half output projection pipelined into the drain: its
                # outT columns have been final since the qb0 blocks' norms,
                # and the pp tiles borrow the just-evacuated avp arena. This
                # keeps the PE busy through the 1/Z chain (no HAM throttle
                # going into the qb1 projection).
                for m in range(2):
                    ppq = avps.tile([128, QB], fp32, tag="av",
                                    name=f"ppq0{m}")
                    for t in range(2):
                        for k2 in range(2):
                            nc.tensor.matmul(
                                ppq[:, ts(t, 512)],
                                wb[("p", k2)][:, ts(m, 128)],
                                outT[k2][:, ts(t, 512)],
                                start=(k2 == 0), stop=(k2 == 1),
                            )
                    ysb = ysp.tile([128, QB], fp32, tag="ysb")
                    nc.vector.tensor_tensor(
                        ysb[:], ppq[:], xb2[m][:, 0:QB], op=OP.add
                    )
                    nc.sync.dma_start(yo[ts(m, 128), 0:QB], ysb[:])
                emit_norm(carry_meta + (oa, rzc))
                # qb1-half projection immediately after the final normalize,
                # still through the borrowed avp arena: the PE never idles
                # (no HAM throttle) and no separate PSUM pool that would
                # alias-wait on the attention pools.
                for m in range(2):
                    ppq = avps.tile([128, QB], fp32, tag="av",
                                    name=f"ppq1{m}")
                    for t in (2, 3):
                        for k2 in range(2):
                            nc.tensor.matmul(
                                ppq[:, ts(t - 2, 512)],
                                wb[("p", k2)][:, ts(m, 128)],
                                outT[k2][:, ts(t, 512)],
                                start=(k2 == 0), stop=(k2 == 1),
                            )
                    ysb = ysp.tile([128, QB], fp32, tag="ysb")
                    nc.vector.tensor_tensor(
                        ysb[:], ppq[:], xb2[m][:, ds(QB, QB)], op=OP.add
                    )
                    nc.sync.dma_start(yo[ts(m, 128), ds(QB, QB)], ysb[:])

        # (output projection fully emitted inside the attention scope)

    nc.compile()
    return nc


def _get_nc():
    if "nc" not in _cached:
        _cached["nc"] = _build_nc()
    return _cached["nc"]


def make_in_maps(x, gn_weight, gn_bias, wq, bq, wk, bk, wv, bv, wp, bp):
    """Per-core input dicts. Core 2*b+half handles batch b, query half `half`;
    its xb has the local token half first (attention is permutation-invariant
    over keys, so key order doesn't matter)."""
    f32 = np.float32
    x = np.asarray(x, f32).reshape(B, C, NTOK)
    # softmax rows sum to 1, so attn @ (V + bv) = attn @ V + bv; folding
    # wp @ bv into the output-projection bias removes the V bias on-device.
    bp_eff = np.asarray(bp, f32) + np.asarray(wp, f32) @ np.asarray(bv, f32)
    base = dict(
        wqT=np.ascontiguousarray(np.asarray(wq, f32).T),
        wkT=np.ascontiguousarray(np.asarray(wk, f32).T),
        wvT=np.ascontiguousarray(np.asarray(wv, f32).T),
        wpT=np.ascontiguousarray(np.asarray(wp, f32).T),
        bqc=np.asarray(bq, f32).reshape(C, 1),
        bkc=np.asarray(bk, f32).reshape(C, 1),
        bpc2=bp_eff.reshape(C, 1),
        gnw=np.asarray(gn_weight, f32).reshape(C, 1),
        gnb=np.asarray(gn_bias, f32).reshape(C, 1),
    )
    in_maps = []
    for core in range(8):
        b, half = core // 2, core % 2
        xbv = x[b]
        if half == 1:
            xbv = np.concatenate([xbv[:, NLOC:], xbv[:, :NLOC]], axis=1)
        m = dict(base)
        m["xb"] = np.ascontiguousarray(xbv)
        in_maps.append(m)
    return in_maps


def assemble_output(results):
    out = np.empty((B, C, NTOK), np.float32)
    for core in range(8):
        b, half = core // 2, core % 2
        out[b][:, half * NLOC : (half + 1) * NLOC] = results[core]["y"]
    return out.reshape(B, C, H, W)


def kernel(x, gn_weight, gn_bias, wq, bq, wk, bk, wv, bv, wp, bp):
    from concourse.bass_utils import run_bass_kernel_spmd

    nc = _get_nc()
    in_maps = make_in_maps(
        x, gn_weight, gn_bias, wq, bq, wk, bk, wv, bv, wp, bp
    )
    res = run_bass_kernel_spmd(nc, in_maps, list(range(8)))
    return assemble_output(res.results)


# revision 36
# speedup vs baseline: 1.0278x; 1.0278x over previous
"""AttentionBlock Trainium2 Bass kernel.

Full-input contract: kernel(**inputs) takes the complete tensors from
setup_inputs() and returns the full (4, 256, 64, 64) float32 output.

Sharding: 8 cores = 4 batches x 2 query-token halves. Each core:
  - group-norms its batch image (stats over all 4096 tokens),
  - computes k, v for all tokens, q for its 2048 local tokens,
  - attention (4 heads, exact softmax without max-subtraction: scores ~N(0,1)),
  - output projection + residual for its token half.
Host side only slices/concats (token order within a batch is rolled so the
local half is always first -> the same SPMD program runs on every core).

Schedule notes (from NTFF traces): the steady-state attention loop is
ACT-bound (exp of 33.5M scores/core at ~1 elem/cycle/lane). Everything else
is arranged to keep the exp stream start as early as possible and the
pre/post phases lean:
  - GN stats split across ACT (sumsq via Square+accum) and DVE (sum via
    tensor_reduce) so the two passes run concurrently.
  - K/Q projections (needed by the first scores) come before V.
  - V bias is folded into the host-precomputed bp' = bp + wp @ bv (softmax
    rows sum to 1), so V-proj is 2 matmuls/chunk.
  - Per-(qb,h) normalization happens inside the loop (the 1/Z broadcast
    matmul borrows the avp PSUM bank right after it is evacuated), so the
    tail is just the output projection.
"""

import sys

sys.path.insert(0, "/opt/trn_rl_repo")

import numpy as np

# hardcoded problem geometry
B, C, H, W = 4, 256, 64, 64
NTOK = H * W            # 4096 keys per image
NLOC = NTOK // 2        # 2048 queries per core
QB = 1024               # query block (scores psum tile free size)
HEADS, D = 4, 64
GROUPS, CPG = 8, 32     # 8 groups x 32 channels
EPS = 1e-5
NCH = 32                # key chunks of 128
VSEG = HEADS * (D + 1)  # 260: per-chunk stride in the VT buffer

_cached = {}


def _build_nc():
    import concourse.mybir as mybir
    import concourse.tile as tile
    from concourse import bacc
    from concourse.bass import ds, ts

    fp32 = mybir.dt.float32
    bf16 = mybir.dt.bfloat16
    AF = mybir.ActivationFunctionType
    OP = mybir.AluOpType
    AX = mybir.AxisListType

    nc = bacc.Bacc("TRN2", target_bir_lowering=False, debug=False, num_devices=8)

    xb = nc.dram_tensor("xb", [C, NTOK], fp32, kind="ExternalInput").ap()
    wqT = nc.dram_tensor("wqT", [C, C], fp32, kind="ExternalInput").ap()
    wkT = nc.dram_tensor("wkT", [C, C], fp32, kind="ExternalInput").ap()
    wvT = nc.dram_tensor("wvT", [C, C], fp32, kind="ExternalInput").ap()
    wpT = nc.dram_tensor("wpT", [C, C], fp32, kind="ExternalInput").ap()
    bqc = nc.dram_tensor("bqc", [C, 1], fp32, kind="ExternalInput").ap()
    bkc = nc.dram_tensor("bkc", [C, 1], fp32, kind="ExternalInput").ap()
    bpc2 = nc.dram_tensor("bpc2", [C, 1], fp32, kind="ExternalInput").ap()
    gnw = nc.dram_tensor("gnw", [C, 1], fp32, kind="ExternalInput").ap()
    gnb = nc.dram_tensor("gnb", [C, 1], fp32, kind="ExternalInput").ap()
    yo = nc.dram_tensor("y", [C, NLOC], fp32, kind="ExternalOutput").ap()

    from contextlib import ExitStack

    with tile.TileContext(nc) as tc, ExitStack() as ctx:
        pool = lambda name, bufs: ctx.enter_context(tc.tile_pool(name=name, bufs=bufs))
        # whole-kernel pools
        consts = pool("consts", 1)
        otp = pool("ot", 1)       # outT (2 x 4KB)
        xkp = pool("xk", 1)       # kept x tiles for the residual (2 x 8KB)
        xb2p = pool("xb2", 1)     # x + bp' residual base (2 x 8KB)

        # ---- x loads first (critical path), split for queue parallelism ----
        xh = {}
        for c2 in range(2):
            for hf in range(2):
                if hf == 0:
                    t = xkp.tile([128, NLOC], fp32, tag=f"xk{c2}",
                                 name=f"x{c2}h{hf}")
                else:
                    t = consts.tile([128, NLOC], fp32, tag=f"xt{c2}",
                                    name=f"x{c2}h{hf}")
                for q4 in range(4):
                    nc.sync.dma_start(
                        t[:, ds(q4 * 512, 512)],
                        xb[ts(c2, 128), ds(hf * NLOC + q4 * 512, 512)],
                    )
                xh[(c2, hf)] = t

        # ---- constants / weights ----
        ones_row = consts.tile([1, NLOC], bf16, tag="ones_row")
        nc.gpsimd.memset(ones_row[:], 1.0)
        ones_col = consts.tile([1, 128], bf16, tag="ones_col")
        nc.gpsimd.memset(ones_col[:], 1.0)
        eps4 = consts.tile([4, 1], fp32, tag="eps4")
        nc.gpsimd.memset(eps4[:], EPS)
        ones64f = consts.tile([1, D], fp32, tag="ones64f")
        nc.gpsimd.memset(ones64f[:], 1.0)
        # ones row AT partition 64: lhsT for the 1/Z broadcast matmul (the
        # Z row lives at partition D of oa/rz tiles; contraction partitions
        # of lhsT and rhs must match)
        onesP = consts.tile([D + 1, D], bf16, tag="onesP")
        nc.gpsimd.memset(onesP[:], 1.0)
        # mask4T[p, gl] = 1/(32*4096) if p//32 == gl: turns per-channel SUM
        # columns into per-group MEANs via one matmul
        mask4T = consts.tile([128, 4], fp32, tag="mask4T")
        nc.gpsimd.memset(mask4T[:], 0.0)
        for gl in range(4):
            nc.gpsimd.memset(
                mask4T[gl * CPG : (gl + 1) * CPG, gl : gl + 1],
                1.0 / (CPG * NTOK),
            )
        # mask4B[gl, p] = 1.0 if p//32 == gl  (group->channel broadcast);
        # row gl>0 starts at partition gl, which memset can't address -> DMA
        # a constant-1.0 fp32 row into place instead.
        mask4B = consts.tile([4, 128], fp32, tag="mask4B")
        nc.gpsimd.memset(mask4B[:], 0.0)
        for gl in range(4):
            nc.sync.dma_start(
                mask4B[gl : gl + 1, gl * CPG : (gl + 1) * CPG],
                ones64f[0:1, 0:CPG],
            )

        bcols = {}
        for nm, src in (("q", bqc), ("k", bkc), ("gw", gnw), ("gb", gnb),
                        ("p2", bpc2)):
            for k2 in range(2):
                t = consts.tile([128, 1], fp32, tag=f"b{nm}{k2}")
                nc.sync.dma_start(t[:], src[ts(k2, 128), :])
                bcols[(nm, k2)] = t

        # weights to bf16 (DVE idle at startup)
        wb = {}
        with tc.tile_pool(name="wload", bufs=4) as wldp:
            for nm, srcw in (("k", wkT), ("q", wqT), ("v", wvT), ("p", wpT)):
                for k2 in range(2):
                    t = wldp.tile([128, C], fp32, tag="wf",
                                  name=f"wf_{nm}{k2}")
                    nc.sync.dma_start(t[:], srcw[ts(k2, 128), :])
                    tb = consts.tile([128, C], bf16, tag=f"w{nm}b{k2}",
                                     name=f"w{nm}b{k2}")
                    nc.vector.tensor_copy(tb[:], t[:])
                    wb[(nm, k2)] = tb

        # VT: per key-chunk j, per head h: [vT(128,64) | ones] at col j*260+h*65
        VT = consts.tile([128, NCH * VSEG], bf16, tag="VT")
        vt_ones = VT[:].rearrange("p (j h x) -> p j h x", j=NCH, h=HEADS)[
            :, :, :, D : D + 1
        ]
        nc.gpsimd.memset(vt_ones, 1.0)

        with tc.tile_pool(name="kq", bufs=1) as kqpool:
            ksb = [kqpool.tile([128, NTOK], bf16, tag=f"ksb{m}", name=f"ksb{m}")
                   for m in range(2)]
            qsb = [kqpool.tile([128, NLOC], bf16, tag=f"qsb{m}", name=f"qsb{m}")
                   for m in range(2)]
            ksw = [kqpool.tile([128, NTOK], bf16, tag=f"ksw{m}", name=f"ksw{m}")
                   for m in range(2)]
            qsw = [kqpool.tile([128, NLOC], bf16, tag=f"qsw{m}", name=f"qsw{m}")
                   for m in range(2)]
            with tc.tile_pool(name="xn", bufs=1) as xnpool:
                xn = [xnpool.tile([128, NTOK], bf16, tag=f"xn{c2}", name=f"xn{c2}")
                      for c2 in range(2)]

                # ---- group-norm: sumsq on ACT (Square+accum), sum on DVE
                # (tensor_reduce) so both passes run concurrently.
                with tc.tile_pool(name="stat", bufs=2) as statp, \
                     tc.tile_pool(name="gnps", bufs=2, space="PSUM") as gnps:
                    for c2 in range(2):
                        sacc = statp.tile([128, 4], fp32, tag="sacc")
                        scr = statp.tile([128, NLOC], bf16, tag="scr", bufs=1)
                        for hf in range(2):
                            nc.scalar.activation(
                                scr[:], xh[(c2, hf)][:], AF.Square,
                                accum_out=sacc[:, 2 + hf : 3 + hf],
                            )
                            nc.vector.tensor_reduce(
                                sacc[:, hf : hf + 1], xh[(c2, hf)][:],
                                axis=AX.X, op=OP.add,
                            )
                        # me2: [sum_p, sumsq_p] (mask4T folds the 1/N)
                        me2 = statp.tile([128, 2], fp32, tag="me2")
                        nc.vector.tensor_add(
                            me2[:, 0:1], sacc[:, 0:1], sacc[:, 1:2]
                        )
                        nc.vector.tensor_add(
                            me2[:, 1:2], sacc[:, 2:3], sacc[:, 3:4]
                        )
                        # group [mean, E[x^2]] onto partitions 0-3 via mask MM
                        gmp = gnps.tile([4, 2], fp32, tag="gmp")
                        nc.tensor.matmul(gmp[:], mask4T[:], me2[:])
                        gmsb = statp.tile([4, 2], fp32, tag="gmsb")
                        nc.vector.tensor_copy(gmsb[:], gmp[:])
                        gvar = statp.tile([4, 1], fp32, tag="gvar")
                        nc.vector.tensor_tensor(
                            gvar[:], gmsb[:, 0:1], gmsb[:, 0:1], op=OP.mult
                        )
                        nc.vector.tensor_tensor(
                            gvar[:], gmsb[:, 1:2], gvar[:], op=OP.subtract
                        )
                        gstd = statp.tile([4, 1], fp32, tag="gstd")
                        nc.scalar.activation(gstd[:], gvar[:], AF.Sqrt,
                                             bias=eps4[:])
                        grstd = statp.tile([4, 1], fp32, tag="grstd")
                        nc.vector.reciprocal(grstd[:], gstd[:])
                        # broadcast group stats back to channel columns
                        rcolp = gnps.tile([128, 1], fp32, tag="rcolp")
                        nc.tensor.matmul(rcolp[:], mask4B[:], grstd[:])
                        mcolp = gnps.tile([128, 1], fp32, tag="mcolp")
                        nc.tensor.matmul(mcolp[:], mask4B[:], gmsb[:, 0:1])
                        acol = statp.tile([128, 1], fp32, tag="acol")
                        nc.vector.tensor_tensor(
                            acol[:], rcolp[:], bcols[("gw", c2)][:], op=OP.mult
                        )
                        bcol = statp.tile([128, 1], fp32, tag="bcol")
                        nc.vector.tensor_tensor(
                            bcol[:], mcolp[:], acol[:], op=OP.mult
                        )
                        nc.vector.tensor_tensor(
                            bcol[:], bcols[("gb", c2)][:], bcol[:], op=OP.subtract
                        )
                        for hf in range(2):
                            nc.vector.tensor_scalar(
                                xn[c2][:, ds(hf * NLOC, NLOC)], xh[(c2, hf)][:],
                                acol[:], bcol[:], op0=OP.mult, op1=OP.add,
                            )
                        if c2 == 1:
                            # preload the exp table set in ACT's idle window
                            # (after the last Sqrt evicted it) so the first
                            # real exp doesn't pay the ~2.7us table load on
                            # the attention critical path
                            dmy = statp.tile([4, 1], fp32, tag="dmy")
                            nc.scalar.activation(dmy[:], eps4[:], AF.Exp)

                # ---- k, q projections first (they gate the first scores),
                # then the swapped copies, then v. The PSUM->SBUF move with
                # bias runs on ACT (Identity, per-partition bias) -- DVE is
                # the pre-phase bottleneck, ACT is idle here.
                with tc.tile_pool(name="qkps", bufs=2, space="PSUM") as qkps:
                    for m in range(2):
                        for t in range(4):
                            pk = qkps.tile([128, 1024], fp32, tag="pk")
                            for half in range(2):
                                for k2 in range(2):
                                    nc.tensor.matmul(
                                        pk[:, ds(half * 512, 512)],
                                        wb[("k", k2)][:, ts(m, 128)],
                                        xn[k2][:, ds(t * 1024 + half * 512, 512)],
                                        start=(k2 == 0), stop=(k2 == 1),
                                    )
                            nc.scalar.activation(
                                ksb[m][:, ds(t * 1024, 1024)], pk[:],
                                AF.Identity, bias=bcols[("k", m)][:],
                            )
                        for t in range(2):
                            pq = qkps.tile([128, 1024], fp32, tag="pk")
                            for half in range(2):
                                for k2 in range(2):
                                    nc.tensor.matmul(
                                        pq[:, ds(half * 512, 512)],
                                        wb[("q", k2)][:, ts(m, 128)],
                                        xn[k2][:, ds(t * 1024 + half * 512, 512)],
                                        start=(k2 == 0), stop=(k2 == 1),
                                    )
                            nc.scalar.activation(
                                qsb[m][:, ds(t * 1024, 1024)], pq[:],
                                AF.Identity, bias=bcols[("q", m)][:],
                            )
                        nc.sync.dma_start(ksw[m][0:64, :], ksb[m][64:128, :])
                        nc.sync.dma_start(ksw[m][64:128, :], ksb[m][0:64, :])
                        nc.sync.dma_start(qsw[m][0:64, :], qsb[m][64:128, :])
                        nc.sync.dma_start(qsw[m][64:128, :], qsb[m][0:64, :])
                    # v projection (bias folded into bp' on the host)
                    for j in range(NCH):
                        pv = qkps.tile([128, C], fp32, tag="pv", bufs=2)
                        nc.tensor.matmul(
                            pv[:], xn[0][:, ts(j, 128)], wb[("v", 0)][:],
                            start=True, stop=False,
                        )
                        nc.tensor.matmul(
                            pv[:], xn[1][:, ts(j, 128)], wb[("v", 1)][:],
                            start=False, stop=True,
                        )
                        dst = VT[:, ds(j * VSEG, VSEG)].rearrange(
                            "p (h x) -> p h x", h=HEADS
                        )[:, :, 0:D]
                        nc.vector.tensor_copy(
                            dst, pv[:].rearrange("p (h x) -> p h x", h=HEADS)
                        )

            # ---- attention (ACT-bound steady state) ----
            outT = [otp.tile([128, NLOC], bf16, tag=f"outT{m}", name=f"outT{m}")
                    for m in range(2)]
            with tc.tile_pool(name="esc", bufs=6) as escp, \
                 tc.tile_pool(name="oa", bufs=2) as oap, \
                 tc.tile_pool(name="rzp", bufs=2) as rzp, \
                 tc.tile_pool(name="tmpn", bufs=2) as tmpp, \
                 tc.tile_pool(name="ys", bufs=2) as ysp, \
                 tc.tile_pool(name="scps", bufs=3, space="PSUM") as scps, \
                 tc.tile_pool(name="avps", bufs=1, space="PSUM") as avps:
                def emit_av(avp, h, j, esc):
                    for t in range(2):
                        nc.tensor.matmul(
                            avp[:, ts(t, 512)],
                            VT[:, ds(j * VSEG + h * (D + 1), D + 1)],
                            esc[:, ts(t, 512)],
                            start=(j == 0), stop=(j == NCH - 1),
                        )

                def emit_norm(dn):
                    # 1/Z broadcast + scale for a finished block; emitted a
                    # few chunk-pairs into the NEXT block so the PE / ACT
                    # streams never stall at the block boundary. The zbc
                    # broadcast tile borrows a scps slot (its previous S
                    # tile's exp is long done by now).
                    dth, dhp, dqb, doa, drzc = dn
                    dzbc = scps.tile([D, QB], fp32, tag="sc", name="zbc")
                    for t in range(2):
                        nc.tensor.matmul(
                            dzbc[:, ts(t, 512)], ones_col[0:1, 0:D],
                            drzc[0:1, ds(t * 512, 512)],
                        )
                    if dhp == 0:
                        nc.vector.tensor_tensor(
                            outT[dth][0:D, ds(dqb * QB, QB)], doa[0:D, :],
                            dzbc[:], op=OP.mult,
                        )
                    else:
                        tm = tmpp.tile([D, QB], bf16, tag="tm")
                        nc.vector.tensor_tensor(tm[:], doa[0:D, :], dzbc[:],
                                                op=OP.mult)
                        nc.sync.dma_start(
                            outT[dth][64:128, ds(dqb * QB, QB)], tm[:]
                        )

                deferred = None
                carry = None
                for qb in range(2):
                    for h in range(HEADS):
                        th, hp = h // 2, h % 2
                        # block 0 only: filler tiles allocated BEFORE avp so
                        # the bufs=1 arena rotation stays in usage order;
                        # their matmuls are emitted into the first-iteration
                        # bubbles (PE waiting on the very first exps), which
                        # otherwise trip the HAM clock gate into half-rate
                        # for the next ~60us.
                        fillers = []
                        if qb == 0 and h == 0:
                            fillers = [
                                avps.tile([D + 1, QB], fp32, tag="av",
                                          name=f"fill{f}")
                                for f in range(4)
                            ]
                        avp = avps.tile([D + 1, QB], fp32, tag="av",
                                        name=f"avp{qb}{h}")
                        pending = []  # (j, esc) awaiting A@V matmuls
                        for jj in range(0, NCH, 2):
                            # chunk pair: even chunk from ksb/qsb at rows
                            # hp*64, odd chunk from the swapped copies at the
                            # OTHER row group -> the four score matmuls run
                            # concurrently in disjoint 64-row halves.
                            b0 = hp * 64
                            b1 = 64 - b0
                            S0 = scps.tile([128, QB], fp32, tag="sc",
                                           name="S0")
                            S1 = scps.tile([128, QB], fp32, tag="sc",
                                           name="S1")
                            # alternate the two 64-row groups every matmul so
                            # consecutive MMs touch disjoint array rows (row
                            # tiling can overlap their streams)
                            for t in range(2):
                                nc.tensor.matmul(
                                    S0[:, ts(t, 512)],
                                    ksb[th][b0 : b0 + 64, ts(jj, 128)],
                                    qsb[th][b0 : b0 + 64,
                                            ds(qb * QB + t * 512, 512)],
                                )
                                nc.tensor.matmul(
                                    S1[:, ts(t, 512)],
                                    ksw[th][b1 : b1 + 64, ts(jj + 1, 128)],
                                    qsw[th][b1 : b1 + 64,
                                            ds(qb * QB + t * 512, 512)],
                                )
                            # (t loop already alternates S0/S1 row groups)
                            esc0 = escp.tile([128, QB], bf16, tag="esc",
                                             name="esc0")
                            nc.scalar.activation(esc0[:], S0[:], AF.Exp,
                                                 scale=0.125)
                            esc1 = escp.tile([128, QB], bf16, tag="esc",
                                             name="esc1")
                            nc.scalar.activation(esc1[:], S1[:], AF.Exp,
                                                 scale=0.125)
                            # A@V lags two pairs behind: by the time the PE
                            # reaches these, their exps finished long ago, so
                            # the in-order PE queue never blocks on ACT.
                            if fillers and jj == 0:
                                # one dense ~8-MM burst right at attention
                                # entry: mirrors the block-boundary burst
                                # that reliably flips the HAM clock gate to
                                # full rate (small 2-MM fillers did not)
                                for ft in fillers:
                                    for t in range(2):
                                        nc.tensor.matmul(
                                            ft[:, ts(t, 512)],
                                            VT[:, 0 : D + 1],
                                            xn[0][:, ts(t, 512)],
                                        )
                            if jj == 0 and carry is not None:
                                # previous block's last two AV pairs: their
                                # exps completed while this block's first
                                # scores ran, so the PE never idles on them.
                                cavp, ch, cpend = carry
                                for pj, pesc in cpend:
                                    emit_av(cavp, ch, pj, pesc)
                                oa = oap.tile([D + 1, QB], fp32, tag="oa")
                                nc.vector.tensor_copy(oa[:], cavp[:])
                                zrow = rzp.tile([1, QB], fp32, tag="zrow",
                                                bufs=2)
                                nc.sync.dma_start(zrow[:], oa[D : D + 1, :])
                                rzf = rzp.tile([1, QB], fp32, tag="rzf",
                                               bufs=2)
                                nc.vector.reciprocal_approx_fast(
                                    rzf[:], zrow[:]
                                )
                                rzc = rzp.tile([1, QB], bf16, tag="rzc",
                                               bufs=2)
                                nc.vector.tensor_copy(rzc[:], rzf[:])
                                deferred = carry_meta + (oa, rzc)
                                carry = None
                            if jj == 8 and deferred is not None:
                                emit_norm(deferred)
                                deferred = None
                            if len(pending) >= 4:
                                for pj, pesc in pending[:2]:
                                    emit_av(avp, h, pj, pesc)
                                pending = pending[2:]
                            pending += [(jj, esc0), (jj + 1, esc1)]
                        # keep the last two pairs for the next block
                        carry = (avp, h, pending)
                        carry_meta = (th, hp, qb)
                    if qb == 0:
                        # residual base x + bp' computed in DVE slack during
                        # the qb=1 attention block
                        xb2 = []
                        for m in range(2):
                            x2 = xb2p.tile([128, NLOC], fp32, tag=f"xb2{m}",
                                           name=f"xb2{m}")
                            nc.vector.tensor_scalar_add(
                                x2[:], xh[(m, 0)][:], bcols[("p2", m)][:]
                            )
                            xb2.append(x2)
                # drain the last block: carried AV pairs, 1/Z, normalize
                cavp, ch, cpend = carry
                for pj, pesc in cpend:
                    emit_av(cavp, ch, pj, pesc)
                oa = oap.tile([D + 1, QB], fp32, tag="oa")
                nc.vector.tensor_copy(oa[:], cavp[:])
                zrow = rzp.tile([1, QB], fp32, tag="zrow", bufs=2)
                nc.sync.dma_start(zrow[:], oa[D : D + 1, :])
                rzf = rzp.tile([1, QB], fp32, tag="rzf", bufs=2)
                nc.vector.reciprocal_approx_fast(rzf[:], zrow[:])
                rzc = rzp.tile([1, QB], bf16, tag="rzc", bufs=2)
                nc.vector.tensor_copy(rzc[:], rzf[:])
                # qb0-half output projection pipelined into the drain: its
                # outT columns have been final since the qb0 blocks' norms,
                # and the pp tiles borrow the just-evacuated avp arena. This
                # keeps the PE busy through the 1/Z chain (no HAM throttle
                # going into the qb1 projection).
                for m in range(2):
                    ppq = avps.tile([128, QB], fp32, tag="av",
                                    name=f"ppq0{m}")
                    for t in range(2):
                        for k2 in range(2):
                            nc.tensor.matmul(
                                ppq[:, ts(t, 512)],
                                wb[("p", k2)][:, ts(m, 128)],
                                outT[k2][:, ts(t, 512)],
                                start=(k2 == 0), stop=(k2 == 1),
                            )
                    ysb = ysp.tile([128, QB], fp32, tag="ysb")
                    nc.vector.tensor_tensor(
                        ysb[:], ppq[:], xb2[m][:, 0:QB], op=OP.add
                    )
                    nc.sync.dma_start(yo[ts(m, 128), 0:QB], ysb[:])
                emit_norm(carry_meta + (oa, rzc))

        # ---- qb1-half output projection + residual (qb0 was emitted inside
        # the attention scope, overlapping the drain) ----
        with tc.tile_pool(name="ysb", bufs=2) as ypool, \
             tc.tile_pool(name="pjps", bufs=2, space="PSUM") as pjps:
            for m in range(2):
                pp = pjps.tile([128, QB], fp32, tag="pp")
                for t in (2, 3):
                    for k2 in range(2):
                        nc.tensor.matmul(
                            pp[:, ts(t - 2, 512)],
                            wb[("p", k2)][:, ts(m, 128)],
                            outT[k2][:, ts(t, 512)],
                            start=(k2 == 0), stop=(k2 == 1),
                        )
                ysb = ypool.tile([128, QB], fp32, tag="ysb")
                nc.vector.tensor_tensor(
                    ysb[:], pp[:], xb2[m][:, ds(QB, QB)], op=OP.add
                )
                nc.sync.dma_start(yo[ts(m, 128), ds(QB, QB)], ysb[:])

    nc.compile()
    return nc


def _get_nc():
    if "nc" not in _cached:
        _cached["nc"] = _build_nc()
    return _cached["nc"]


def make_in_maps(x, gn_weight, gn_bias, wq, bq, wk, bk, wv, bv, wp, bp):
    """Per-core input dicts. Core 2*b+half handles batch b, query half `half`;
    its xb has the local token half first (attention is permutation-invariant
    over keys, so key order doesn't matter)."""
    f32 = np.float32
    x = np.asarray(x, f32).reshape(B, C, NTOK)
    # softmax rows sum to 1, so attn @ (V + bv) = attn @ V + bv; folding
    # wp @ bv into the output-projection bias removes the V bias on-device.
    bp_eff = np.asarray(bp, f32) + np.asarray(wp, f32) @ np.asarray(bv, f32)
    base = dict(
        wqT=np.ascontiguousarray(np.asarray(wq, f32).T),
        wkT=np.ascontiguousarray(np.asarray(wk, f32).T),
        wvT=np.ascontiguousarray(np.asarray(wv, f32).T),
        wpT=np.ascontiguousarray(np.asarray(wp, f32).T),
        bqc=np.asarray(bq, f32).reshape(C, 1),
        bkc=np.asarray(bk, f32).reshape(C, 1),
        bpc2=bp_eff.reshape(C, 1),
        gnw=np.asarray(gn_weight, f32).reshape(C, 1),
        gnb=np.asarray(gn_bias, f32).reshape(C, 1),
    )
    in_maps = []
    for core in range(8):
        b, half = core // 2, core % 2
        xbv = x[b]
        if half == 1:
            xbv = np.concatenate([xbv[:, NLOC:], xbv[:, :NLOC]], axis=1)
        m = dict(base)
        m["xb"] = np.ascontiguousarray(xbv)
        in_maps.append(m)
    return in_maps


def assemble_output(results):
    out = np.empty((B, C, NTOK), np.float32)
    for core in range(8):
        b, half = core // 2, core % 2
        out[b][:, half * NLOC : (half + 1) * NLOC] = results[core]["y"]
    return out.reshape(B, C, H, W)


def kernel(x, gn_weight, gn_bias, wq, bq, wk, bk, wv, bv, wp, bp):
    from concourse.bass_utils import run_bass_kernel_spmd

    nc = _get_nc()
    in_maps = make_in_maps(
        x, gn_weight, gn_bias, wq, bq, wk, bk, wv, bv, wp, bp
    )
    res = run_bass_kernel_spmd(nc, in_maps, list(range(8)))
    return assemble_output(res.results)
